# revision 37
# baseline (speedup 1.0000x reference)
"""DirectionalConv3d Trainium2 kernel — pack-T2 layout, bf16 I/O.

out[b, o, t, r, c] = sum_d W_d[o, :] . x[b, :, (t,r,c)+delta_d]
for the 7-point directional stencil (self, t+-1, r+-1, c+-1), zero padded.

Strategy (1 batch per core, 8 cores):
  - Host casts x to bf16 and packs plane-parity onto partition halves:
    partition p<64 holds channel p of EVEN t-planes, partition 64+p holds
    channel p of ODD t-planes ("superplane" u = plane pair (2u, 2u+1),
    free dim = u*1024 + r*32 + c).  Output uses the same packed layout
    (psum partitions 0-63 = out[2u] channels, 64-127 = out[2u+1]) and is
    written back as bf16; the host unpacks and casts to f32.
  - Per superplane, 7 stencil directions collapse into:
      * one dense K=128 "A" matmul  lhsT=[[Wself,Wtp],[Wtm,Wself]]
        covering self(both planes) + tp/tm INSIDE the pair at full
        16384-MAC/cycle array efficiency,
      * two K=64 off-diagonal quadrant matmuls for the cross-pair t terms
        (out[2u] += Wtp x[2u-1], out[2u+1] += Wtm x[2u+2]) which stream
        at 2 bf16 cols/cycle,
      * four K=128 block-diagonal matmuls diag(Wd, Wd) for r+-1 / c+-1
        (same spatial shift for both plane parities).  r shifts are
        contiguous-AP row trims; c shifts use 2-D strided APs
        [16 rows, 31 cols] so no padding and no wrap corrections exist.
    This halves PE column-issues vs the all-K=64 formulation and the
    bf16 I/O halves HBM traffic vs f32 (the two former co-bottlenecks).
  - PSUM: 2 banks per superplane, 8-bank rotation (4 superplanes in
    flight).  VectorE/ScalarE evacuate psum f32 -> bf16 staging, DMA out.
"""

import numpy as np
import ml_dtypes
import os

B = 8
CI = 64
CO = 64
T = 32
R = 32
C = 32
U = T // 2           # 16 superplanes
SPL = R * C          # 1024 elements per (super)plane per partition
NF = U * SPL         # 16384 free elements per partition half
WCOLS = 704          # weight SBUF columns: A | rp | rm | cp | cm | cross

SG = int(os.environ.get("KERNEL_SG", "2"))    # superplanes per output stage
# input chunk sizes in superplanes: small first chunks let the first
# matmul start as soon as ~256 KB has landed instead of ~1 MB.
CHUNKS = [int(c) for c in os.environ.get("KERNEL_CHUNKS",
                                         "1,1,2,2,2,2,2,2,2").split(",")]
assert sum(CHUNKS) == U
_CHUNK_OF = []
for _k, _c in enumerate(CHUNKS):
    _CHUNK_OF += [_k] * _c
_CHUNK_BASE = [sum(CHUNKS[:k]) for k in range(len(CHUNKS))]

# output stage groups: SG superplanes each, but the last two flush singly
_SGROUPS = []
_u = 0
while _u < U:
    _g1 = min(_u + SG, U) if _u < U - 2 else _u + 1
    _SGROUPS.append((_u, _g1))
    _u = _g1
_SGROUP_OF = {}
for _g0, _g1 in _SGROUPS:
    for _uu in range(_g0, _g1):
        _SGROUP_OF[_uu] = (_g0, _g1)

_NC_CACHE = {}


def _emit(nc, tc, x, wt, out, mybir, bass):
    xdt = mybir.dt.bfloat16
    AP = bass.AP

    wpool = tc.alloc_tile_pool(name="wp", bufs=1)
    xpool = tc.alloc_tile_pool(name="xin", bufs=1)
    apool = tc.alloc_tile_pool(name="accp", bufs=7, space="PSUM")
    wmpool = tc.alloc_tile_pool(name="wmp", bufs=1, space="PSUM")
    spool = tc.alloc_tile_pool(name="stg", bufs=4)

    # ---- weights [128, 704] prepacked host-side (see host_weights) ----
    # (the weight DMAs are emitted inside the chunk loop, behind the first
    # two input-hi chunks: the SP ring is FIFO and the first matmul needs
    # those chunks before it needs weights)
    w_sb = wpool.tile([128, WCOLS], xdt, name="w_sb")
    wA = w_sb[:, 0:128]
    wRP = w_sb[:, 128:256]
    wRM = w_sb[:, 256:384]
    wCP = w_sb[:, 384:512]
    wCM = w_sb[:, 512:640]
    wTPx = w_sb[64:128, 640:704]   # cross: out[2u] += Wtp x[2u-1]
    wTMx = w_sb[0:64, 640:704]     # cross: out[2u+1] += Wtm x[2u+2]

    # ---- x image: direct bf16 DMA, no staging, no casts, no memsets ----
    # two 64-partition DMAs per chunk (complementary SDMA engine sets);
    # lo half on SWDGE (gpsimd — its own descriptor path, dodges the
    # HWDGE rings that are busy with the runtime ACT-table preamble),
    # hi half on the SP HWDGE ring.  Descriptor generation runs on two
    # independent engines this way.
    xts = []
    for k, c in enumerate(CHUNKS):
        n = c * SPL
        off = _CHUNK_BASE[k] * SPL
        xt = xpool.tile([128, n], xdt, name=f"xc{k}")
        nc.gpsimd.dma_start(out=xt[0:64, :],
                            in_=AP(x.tensor, off, [[NF, 64], [1, n]]))
        nc.sync.dma_start(out=xt[64:128, :],
                          in_=AP(x.tensor, 64 * NF + off, [[NF, 64], [1, n]]))
        xts.append(xt)
        if k == 1:
            nc.sync.dma_start(out=w_sb[0:64, :],
                              in_=AP(wt.tensor, 0, [[WCOLS, 64], [1, WCOLS]]))
            nc.sync.dma_start(out=w_sb[64:128, :],
                              in_=AP(wt.tensor, 64 * WCOLS, [[WCOLS, 64], [1, WCOLS]]))

    def xv(u, lo, sz, p0=0, p1=128):
        """SBUF AP for packed superplane u, free offset lo, length sz."""
        k = _CHUNK_OF[u]
        base = (u - _CHUNK_BASE[k]) * SPL
        return xts[k][p0:p1, base + lo:base + lo + sz]

    def xvr(u, j, p0=0, p1=128):
        """[p, 16 rows, 32 cols] view of bank j's rows of superplane u."""
        k = _CHUNK_OF[u]
        base = (u - _CHUNK_BASE[k]) * SPL + j * 512
        return xts[k][p0:p1, base:base + 512].rearrange("p (r c) -> p r c", c=C)

    mm = nc.tensor.matmul

    # ---- PE warm-up: dummy matmuls on a locally-memset tile keep the HAM
    # activity monitor busy while the x chunks stream in, so the real
    # matmuls start at 2.4 GHz instead of ramping from 1.2.  PE would
    # otherwise idle through the whole fill phase.  Deliberately NOT on
    # w_sb: a DMA dependency would gate the warm-up on semaphores that
    # fire ~12us in (measured), defeating the point.
    nwarm = int(os.environ.get("KERNEL_WARM", "36"))
    if nwarm:
        wsrc = wpool.tile([128, 128], xdt, name="wsrc")
        nc.vector.memset(wsrc[:, :], 0.0)
        wacc = wmpool.tile([128, 128], mybir.dt.float32, name="wacc")
        for i in range(nwarm):
            mm(out=wacc[:, :], lhsT=wsrc[:, :], rhs=wsrc[:, :],
               start=True, stop=True, skip_group_check=True)

    accs = {}
    stage_ref = [None]

    def emit_k128(u):
        """A + r/c block-diagonal passes (all full-array, no geometry
        switch, LDWEIGHTS hides in the background weight buffer)."""
        a = accs[u]
        for j in range(2):  # A: self(both) + tp/tm internal (dense K=128)
            mm(out=a[j][:, :], lhsT=wA, rhs=xv(u, j * 512, 512),
               start=True, stop=False, skip_group_check=True)
        # r+-1 (block-diagonal K=128, contiguous row-trimmed APs)
        mm(out=a[0][:, 32:512], lhsT=wRP, rhs=xv(u, 0, 480),
           start=False, stop=False, skip_group_check=True)
        mm(out=a[1][:, 0:512], lhsT=wRP, rhs=xv(u, 480, 512),
           start=False, stop=False, skip_group_check=True)
        mm(out=a[0][:, 0:512], lhsT=wRM, rhs=xv(u, 32, 512),
           start=False, stop=False, skip_group_check=True)
        mm(out=a[1][:, 0:480], lhsT=wRM, rhs=xv(u, 544, 480),
           start=False, stop=False, skip_group_check=True)
        # c+-1 (block-diagonal K=128, 2-D strided APs)
        ovs = [a[j][:, :].rearrange("p (r c) -> p r c", c=C) for j in range(2)]
        xrs = [xvr(u, j) for j in range(2)]
        for j in range(2):
            mm(out=ovs[j][:, :, 1:32], lhsT=wCP, rhs=xrs[j][:, :, 0:31],
               start=False, stop=False, skip_group_check=True)
        for j in range(2):
            mm(out=ovs[j][:, :, 0:31], lhsT=wCM, rhs=xrs[j][:, :, 1:32],
               start=False, stop=False, skip_group_check=True)

    def emit_k64(u):
        """Cross-pair t terms (K=64 quadrants), stop on the last per bank."""
        a = accs[u]
        for j in range(2):
            if u > 0:
                mm(out=a[j][0:64, :], lhsT=wTPx,
                   rhs=xv(u - 1, j * 512, 512, 64, 128),
                   start=False, stop=(u == U - 1), skip_group_check=True)
        for j in range(2):
            if u < U - 1:
                mm(out=a[j][64:128, :], lhsT=wTMx,
                   rhs=xv(u + 1, j * 512, 512, 0, 64),
                   start=False, stop=True, skip_group_check=True)

    def emit_evac(u):
        """PSUM -> bf16 staging; DMA out per stage group.  The last two
        superplanes flush individually so the final (unoverlappable) DMA
        is as small as possible.  The two output halves go on DIFFERENT
        HWDGE rings (lo on ACT, hi on SP) so they drain in parallel —
        each ring is strictly FIFO."""
        a = accs[u]
        g0, g1 = _SGROUP_OF[u]
        if u == g0:
            stage_ref[0] = spool.tile([128, (g1 - g0) * SPL], xdt,
                                      name=f"st{u}", tag="st")
        stage = stage_ref[0]
        soff = (u - g0) * SPL
        nc.vector.tensor_copy(out=stage[:, soff:soff + 512], in_=a[0][:, :])
        nc.scalar.copy(out=stage[:, soff + 512:soff + SPL], in_=a[1][:, :])
        if u == g1 - 1:
            n = (g1 - g0) * SPL
            nc.scalar.dma_start(
                out=AP(out.tensor, g0 * SPL, [[NF, 64], [1, n]]),
                in_=stage[0:64, :n])
            nc.sync.dma_start(
                out=AP(out.tensor, 64 * NF + g0 * SPL, [[NF, 64], [1, n]]),
                in_=stage[64:128, :n])

    # Superplanes run in PAIRS: [K128(u), K128(u+1), K64(u), K64(u+1)] so
    # the 128<->64 tile-geometry reconfiguration (~225 ns measured, both
    # directions) is paid once per pair instead of once per superplane.
    # The last two superplanes stay unpaired so their stop matmuls (and
    # hence the final evacuation + DMA) come as early as possible.
    groups = [(u, u + 1) for u in range(0, U - 2, 2)] + [(U - 2,), (U - 1,)]
    for grp in groups:
        for u in grp:
            accs[u] = [apool.tile([128, 512], mybir.dt.float32,
                                  name=f"a{u}_{j}", tag="acc")
                       for j in range(2)]
            emit_k128(u)
        for u in grp:
            emit_k64(u)
        for u in grp:
            emit_evac(u)

    for p in (spool, wmpool, apool, xpool, wpool):
        p.release()


def _split_multi_waits(nc, mybir):
    """Walrus codegen allows only one sem-wait slot per engine instruction
    ("Too many sync wait commands").  Hoist all but one wait of any
    multi-wait instruction onto InstNoOp's inserted immediately before it
    on the same engine queue — semantically identical for in-order
    engines (the nop blocks the queue until its wait passes).
    """
    SyncInfo = mybir.SyncInfo
    counter = [0]
    for blk in nc.m.functions[0].blocks:
        insts = list(blk.instructions)
        out, changed = [], False
        for inst in insts:
            si = getattr(inst, "sync_info", None)
            waits = list(si.on_wait) if si is not None and si.on_wait else []
            if len(waits) > 1:
                for w in waits[:-1]:
                    nop = mybir.InstNoOp(name=f"waitnop_{counter[0]}")
                    counter[0] += 1
                    nop.engine = inst.engine
                    nop.sync_info = SyncInfo(on_wait=[w], on_update=[])
                    nc.register_instruction(nop, overwrite=True)
                    out.append(nop)
                si.on_wait = [waits[-1]]
                changed = True
            out.append(inst)
        if changed:
            blk.instructions = out


def build_nc():
    import concourse.bass as bass
    import concourse.mybir as mybir
    import concourse.tile as tile

    key = (tuple(CHUNKS), SG)
    if key in _NC_CACHE:
        return _NC_CACHE[key]
    nc = bass.Bass("TRN2", target_bir_lowering=False, debug=False)
    x = nc.dram_tensor("x", [128, NF], mybir.dt.bfloat16, kind="ExternalInput").ap()
    wt = nc.dram_tensor("wt", [128, WCOLS], mybir.dt.bfloat16,
                        kind="ExternalInput").ap()
    out = nc.dram_tensor("out", [128, NF], mybir.dt.bfloat16,
                         kind="ExternalOutput").ap()
    with tile.TileContext(nc) as tc:
        _emit(nc, tc, x, wt, out, mybir, bass)
    _split_multi_waits(nc, mybir)
    _NC_CACHE[key] = nc
    return nc


def host_weights(inputs):
    """Pack the 7 64x64 weights into the [128, 704] bf16 lhsT block layout.

    cols 0-127:  A = [[Wself^T, Wtp^T], [Wtm^T, Wself^T]]
    cols 128-639: diag2(Wrp), diag2(Wrm), diag2(Wcp), diag2(Wcm)
    cols 640-703: rows 64-127 = Wtp^T (tp cross), rows 0-63 = Wtm^T (tm cross)
    """
    W = {n: np.asarray(inputs[n], dtype=np.float32)
         for n in ("w_self", "w_tp", "w_tm", "w_rp", "w_rm", "w_cp", "w_cm")}
    wt = np.zeros((128, WCOLS), np.float32)
    wt[0:64, 0:64] = W["w_self"].T
    wt[64:128, 0:64] = W["w_tm"].T
    wt[0:64, 64:128] = W["w_tp"].T
    wt[64:128, 64:128] = W["w_self"].T
    for i, n in enumerate(("w_rp", "w_rm", "w_cp", "w_cm")):
        c0 = 128 + i * 128
        wt[0:64, c0:c0 + 64] = W[n].T
        wt[64:128, c0 + 64:c0 + 128] = W[n].T
    wt[64:128, 640:704] = W["w_tp"].T
    wt[0:64, 640:704] = W["w_tm"].T
    return wt.astype(ml_dtypes.bfloat16)


def host_x(inputs):
    """Per-batch packed bf16 images [128, 16384]: even planes on rows 0-63,
    odd planes on rows 64-127."""
    x = np.asarray(inputs["x"], dtype=np.float32)
    xs = []
    for b in range(B):
        xe = x[b][:, 0::2].reshape(CI, NF)
        xo = x[b][:, 1::2].reshape(CI, NF)
        xs.append(np.ascontiguousarray(
            np.concatenate([xe, xo], axis=0)).astype(ml_dtypes.bfloat16))
    return xs


def host_out(res):
    """Unpack per-core [128, 16384] bf16 results to [B, 64, 32, 32, 32] f32."""
    out = np.empty((B, CO, T, R, C), np.float32)
    for b in range(B):
        o = np.asarray(res[b]["out"]).astype(np.float32).reshape(2, CO, U, R, C)
        out[b, :, 0::2] = o[0]
        out[b, :, 1::2] = o[1]
    return out


def kernel(**inputs):
    from concourse.bass_utils import run_bass_kernel_spmd

    nc = build_nc()
    wt = host_weights(inputs)
    xs = host_x(inputs)
    in_maps = [{"x": xs[b], "wt": wt} for b in range(B)]
    res = run_bass_kernel_spmd(nc, in_maps, list(range(B))).results
    return host_out(res)
